# revision 5
# baseline (speedup 1.0000x reference)
"""Trainium2 Bass kernel for CalibrationFreeFP8Linear.

Computes: quantize x and w to fp8-e4m3 with EMA-updated dynamic absmax
scales, fp8 matmul (fp32 accumulate), dequantize, cast to bf16.

Sharding: data-parallel over the 16384 (B*S) rows of x across 8 cores;
weight replicated. The x absmax needs a global max -> AllReduce(max).

Host side pre-transposes both operands to K-major layout ([K, M] / [K, N])
so the tensor engine (which contracts over the partition axis for both
operands) gets contiguous DMA loads with no on-device transpose.
"""

import numpy as np
import ml_dtypes

import concourse.bass as bass
import concourse.mybir as mybir
import concourse.tile as tile
from concourse import bacc, bass_isa
from concourse.bass import ts
from concourse.bass_utils import run_bass_kernel_spmd

FP8_MAX = 448.0
EMA = 0.9
N_CORES = 8
P = 128

# Full problem shapes (hardcoded; kernel.py must be self-contained).
B, S, K, N = 4, 4096, 2048, 2048
M_PER_CORE = (B * S) // N_CORES  # 2048


def build_nc(M, K, N, n_cores=N_CORES, dma_k=2):
    """Build the SPMD Bass program for one core's [M, K] @ [K, N]^T shard.

    DRAM inputs (per core): xt [K, M] bf16, wt [K, N] bf16 (both K-major),
    in_s [1] f32, w_s [1] f32. Output: out [M, N] bf16.
    """
    dt = mybir.dt
    KT = K // P            # k-subtiles
    MT = M // P            # m-tiles
    N_TILE = min(512, N)
    NT = N // N_TILE
    assert K % P == 0 and M % P == 0 and N % N_TILE == 0
    assert KT % 2 == 0, "DoubleRow needs an even number of k-subtiles"
    assert KT % dma_k == 0
    n_dma = KT // dma_k

    nc = bacc.Bacc(
        "TRN2",
        target_bir_lowering=False,
        debug=False,
        num_devices=n_cores,
    )

    xt = nc.dram_tensor("xt", [K, M], dt.bfloat16, kind="ExternalInput").ap()
    wt = nc.dram_tensor("wt", [K, N], dt.bfloat16, kind="ExternalInput").ap()
    in_s = nc.dram_tensor("in_s", [1], dt.float32, kind="ExternalInput").ap()
    w_s = nc.dram_tensor("w_s", [1], dt.float32, kind="ExternalInput").ap()
    out = nc.dram_tensor("out", [M, N], dt.bfloat16, kind="ExternalOutput").ap()

    # K-major views of the DRAM tensors: k = ko*P + p
    xt_v = xt.rearrange("(ko p) m -> p ko m", p=P)
    wt_v = wt.rearrange("(ko p) n -> p ko n", p=P)
    out_v = out.rearrange("(mo p) n -> p mo n", p=P)

    rg = [list(range(n_cores))]

    with tile.TileContext(nc) as tc:
        with (
            tc.tile_pool(name="const", bufs=1) as const,
            tc.tile_pool(name="stats", bufs=1) as stats,
            tc.tile_pool(name="dram", bufs=1, space="DRAM") as dram,
            tc.tile_pool(name="wb_pool", bufs=1) as wb_pool,
            tc.tile_pool(name="xs_pool", bufs=3) as xs_pool,
            tc.tile_pool(name="wf_pool", bufs=1) as wf_pool,
            tc.tile_pool(name="xf_pool", bufs=1) as xf_pool,
            tc.tile_pool(name="psum", bufs=4, space="PSUM") as psum,
            tc.tile_pool(name="outp", bufs=3) as outp,
        ):
            # ---- load w (resident) + running absmax; x pass 1 (absmax only)
            wb = wb_pool.tile([P, KT, N], dt.bfloat16)
            rmax_w = stats.tile([P, n_dma], dt.float32)
            rmax_x = stats.tile([P, n_dma], dt.float32)
            for j in range(n_dma):
                nc.sync.dma_start(wb[:, ts(j, dma_k)], wt_v[:, ts(j, dma_k)])
                nc.vector.tensor_reduce(
                    rmax_w[:, j : j + 1],
                    wb[:, ts(j, dma_k)],
                    axis=mybir.AxisListType.XY,
                    op=mybir.AluOpType.max,
                    apply_absolute_value=True,
                )
            for j in range(n_dma):
                xs = xs_pool.tile([P, dma_k, M], dt.bfloat16, name="xs")
                nc.sync.dma_start(xs, xt_v[:, ts(j, dma_k)])
                nc.vector.tensor_reduce(
                    rmax_x[:, j : j + 1],
                    xs,
                    axis=mybir.AxisListType.XY,
                    op=mybir.AluOpType.max,
                    apply_absolute_value=True,
                )

            # ---- local absmax scalars (broadcast to all 128 partitions)
            amax_w = stats.tile([P, 1], dt.float32)
            amax_x = stats.tile([P, 1], dt.float32)
            nc.vector.tensor_reduce(
                amax_w, rmax_w, axis=mybir.AxisListType.X, op=mybir.AluOpType.max
            )
            nc.vector.tensor_reduce(
                amax_x, rmax_x, axis=mybir.AxisListType.X, op=mybir.AluOpType.max
            )
            amax_w_b = stats.tile([P, 1], dt.float32)
            amax_x_b = stats.tile([P, 1], dt.float32)
            nc.gpsimd.partition_all_reduce(
                amax_w_b, amax_w, channels=P, reduce_op=bass_isa.ReduceOp.max
            )
            nc.gpsimd.partition_all_reduce(
                amax_x_b, amax_x, channels=P, reduce_op=bass_isa.ReduceOp.max
            )

            # ---- global x absmax across cores: AllReduce(max) of a 512B vec
            cc_in = dram.tile([P], dt.float32)
            cc_out = dram.tile([P], dt.float32, addr_space="Shared")
            cc_in_v = cc_in.rearrange("(o p) -> p o", p=P)
            cc_out_v = cc_out.rearrange("(o p) -> p o", p=P)
            nc.sync.dma_start(cc_in_v, amax_x_b)
            if n_cores > 1:
                nc.gpsimd.collective_compute(
                    "AllReduce",
                    mybir.AluOpType.max,
                    replica_groups=rg,
                    ins=[cc_in.opt()],
                    outs=[cc_out.opt()],
                )
                amax_x_g = stats.tile([P, 1], dt.float32)
                nc.sync.dma_start(amax_x_g, cc_out_v)
            else:
                amax_x_g = amax_x_b

            # ---- scales:
            # s = EMA*prev + (1-EMA)*clip(448/(amax+1e-12), 1e-6, 1e6)
            prev_s = stats.tile([1, 2], dt.float32)
            nc.sync.dma_start(prev_s[:, 0:1], in_s.rearrange("(o p) -> p o", p=1))
            nc.sync.dma_start(prev_s[:, 1:2], w_s.rearrange("(o p) -> p o", p=1))
            prev_b = stats.tile([P, 2], dt.float32)
            nc.gpsimd.partition_broadcast(prev_b, prev_s, channels=P)
            in_s_b = prev_b[:, 0:1]
            w_s_b = prev_b[:, 1:2]

            def ema_scale(amax_col, prev_col, name):
                t = stats.tile([P, 1], dt.float32, name=f"t_{name}")
                nc.vector.tensor_scalar_add(t, amax_col, 1e-12)
                nc.vector.reciprocal(t, t)
                nc.vector.tensor_scalar_mul(t, t, FP8_MAX)
                nc.vector.tensor_scalar(
                    t, t, 1e-6, 1e6, mybir.AluOpType.max, mybir.AluOpType.min
                )
                s = stats.tile([P, 1], dt.float32, name=f"s_{name}")
                nc.vector.tensor_scalar_mul(s, t, 1.0 - EMA)
                t2 = stats.tile([P, 1], dt.float32, name=f"t2_{name}")
                nc.vector.tensor_scalar_mul(t2, prev_col, EMA)
                nc.vector.tensor_add(s, s, t2)
                return s

            s_x = ema_scale(amax_x_g, in_s_b, "x")
            s_w = ema_scale(amax_w_b, w_s_b, "w")

            # inv = 1 / (s_x * s_w) for the output dequant
            inv = stats.tile([P, 1], dt.float32)
            nc.vector.tensor_mul(inv, s_x, s_w)
            nc.vector.reciprocal(inv, inv)

            # ---- quantize w (ACT, from resident bf16) and x (DVE, pass 2)
            wf = wf_pool.tile([P, KT, N], dt.float8e4)
            xf = xf_pool.tile([P, KT, M], dt.float8e4)
            for t in range(KT):
                nc.scalar.mul(wf[:, t], wb[:, t], mul=s_w)
            for j in range(n_dma):
                xs2 = xs_pool.tile([P, dma_k, M], dt.bfloat16, name="xs")
                nc.sync.dma_start(xs2, xt_v[:, ts(j, dma_k)])
                nc.vector.tensor_scalar_mul(xf[:, ts(j, dma_k)], xs2, s_x)

            # ---- fp8 DoubleRow matmul + dequant epilogue
            for m in range(MT):
                out_m = outp.tile([P, N], dt.bfloat16, name="out_m")
                for n in range(NT):
                    pt = psum.tile([P, N_TILE], dt.float32, name="pt")
                    for kk in range(KT // 2):
                        nc.tensor.matmul(
                            pt,
                            xf[:, 2 * kk : 2 * kk + 2, ts(m, P)],
                            wf[:, 2 * kk : 2 * kk + 2, ts(n, N_TILE)],
                            start=(kk == 0),
                            stop=(kk == KT // 2 - 1),
                            perf_mode=mybir.MatmulPerfMode.DoubleRow,
                        )
                    nc.scalar.mul(out_m[:, ts(n, N_TILE)], pt, mul=inv)
                nc.sync.dma_start(out_v[:, m], out_m)

    nc.compile()
    return nc


_NC_CACHE = {}


def _get_nc(M, K, N, n_cores=N_CORES):
    key = (M, K, N, n_cores)
    if key not in _NC_CACHE:
        _NC_CACHE[key] = build_nc(M, K, N, n_cores)
    return _NC_CACHE[key]


def run_sharded(x2d, weight, input_scale, weight_scale, n_cores=N_CORES, trace=False):
    """x2d: [rows, K] bf16, weight: [N, K] bf16. Returns ([rows, N] bf16, result)."""
    rows, k = x2d.shape
    n = weight.shape[0]
    m_per = rows // n_cores
    nc = _get_nc(m_per, k, n, n_cores)

    wt = np.ascontiguousarray(weight.T)  # [K, N]
    in_s = np.asarray(input_scale, dtype=np.float32).reshape(1)
    w_s = np.asarray(weight_scale, dtype=np.float32).reshape(1)
    in_maps = []
    for i in range(n_cores):
        xt_i = np.ascontiguousarray(x2d[i * m_per : (i + 1) * m_per].T)  # [K, M]
        in_maps.append({"xt": xt_i, "wt": wt, "in_s": in_s, "w_s": w_s})

    res = run_bass_kernel_spmd(nc, in_maps, core_ids=list(range(n_cores)), trace=trace)
    out = np.concatenate([res.results[i]["out"] for i in range(n_cores)], axis=0)
    return out, res


def kernel(x, weight, input_scale, weight_scale):
    x = np.asarray(x)
    weight = np.asarray(weight)
    b, s, k = x.shape
    x2d = np.ascontiguousarray(x.reshape(b * s, k))
    out, _ = run_sharded(x2d, weight, input_scale, weight_scale)
    return out.reshape(b, s, weight.shape[0]).astype(ml_dtypes.bfloat16)
